# revision 1
# baseline (speedup 1.0000x reference)
"""Trainium2 Bass kernel for a dense transformer block.

Model (B=4, N=1024, D=1024, H=16, hd=64):
  q/k/v = x{q,k,v} @ W{q,k,v}.T ; attn = softmax(mask(q k^T / 8)) @ v
  x1 = LN1(x_q + attn_out @ Wp.T + bp)
  out = LN2(x1 + relu(x1 @ W1.T + bf1) @ W2.T + bf2)

Sharding: 8 cores = (batch b, query-half qh); each core owns 512 queries.

Design (sim: 163us vs 265us for the f32r baseline):
- KV gather: only mask-valid kv columns are shipped/computed (padded to a
  multiple of 128; padding gets a -1e5 score bias so exp()==0 exactly).
  Shrinks K/V projection, scores, softmax, and attn-out by ~3/8 (njv=5).
  The program is compiled per njv (cached); inputs are deterministic.
- bf16 matmul operands (f32 PSUM accumulation): same PE rate as f32r at
  free>=256, half the DMA bytes and SBUF, 2x DVE. Residual/LN inputs
  (xqf, xres, yres) stay f32 to keep max-norm rel err ~4.4e-3 (<2e-2).
- Feature-major layout: activations live as x^T[d, n] with d = mt*128 + p.
  Partition-axis reductions (softmax denom, LN sums) go through the PE via
  ones-vector matmuls / an appended ones column in V.
- Software pipelining: V-half1 projection (quarter units) and 1 column of
  Wp accumulate inside the attention loop (fills PE while Act runs exps);
  5 score-PSUM buffers give the exp pipeline a full head of slack;
  LN sums ride behind the Wp/FFN2 epilogues; FFN1 runs on z = g1*xres with
  the LN1 correction applied algebraically afterwards (W1 x1 = rstd*(W1 z)
  + nmr*(W1 g1) + W1 b1), so the PE never waits on LN1 stats; LN2 runs
  column-split with a pipelined DVE/Pool/Act/2-queue-DMA tail.
- HW legality (BIR verifier): GPSIMD/Pool never touches PSUM (copies and
  LN middles on DVE/Act instead), one-PSUM-operand rule respected, no
  memsets (constants DMA'd from a ones input), rstd = Act-Sqrt of DVE
  reciprocal; sqrt act-table preloaded after attention (2 loads total).
- Weights stream per-128-row-tile on the SP and Act DMA queues; w2 shares
  wv's SBUF (loads during attention); out DMAs overlap the LN2 tail.
"""
import numpy as np

P = 128
DIM = 1024
HEADS = 16
HD = 64
B = 4
NQ = 1024
NKV = 1024
TQ = 512          # queries per core
MT = DIM // P     # 8 feature tiles
SCALE = HD ** -0.5
NEG = -1e5

_CACHE = {}


def _build(njv):
    import concourse.bass as bass
    import concourse.mybir as mybir
    import concourse.tile as tile
    from concourse import bacc

    f32 = mybir.dt.float32
    f32r = mybir.dt.float32r
    bf16 = mybir.dt.bfloat16
    AF = mybir.ActivationFunctionType
    OP = mybir.AluOpType

    nkvv = njv * P

    nc = bacc.Bacc("TRN2", target_bir_lowering=False, debug=False)

    xqT_d = nc.dram_tensor("xqT", [P, MT, TQ], bf16, kind="ExternalInput").ap()
    xqTf_d = nc.dram_tensor("xqTf", [P, MT, TQ], f32r, kind="ExternalInput").ap()
    xkT_d = nc.dram_tensor("xkT", [P, MT, nkvv], bf16, kind="ExternalInput").ap()
    xvT_d = nc.dram_tensor("xvT", [P, MT, nkvv], bf16, kind="ExternalInput").ap()
    wv_d = nc.dram_tensor("wv_r", [P, MT, DIM], bf16, kind="ExternalInput").ap()
    wt_d = {}
    for w in ("wk", "wq", "wp", "w1", "w2"):
        wt_d[w] = nc.dram_tensor(w, [MT, P, MT, P], bf16, kind="ExternalInput").ap()
    mask_d = nc.dram_tensor("maskb", [P, njv], f32, kind="ExternalInput").ap()
    vec_d = {}
    for v in ("bp", "bf1", "bf2", "g1", "b1", "g2", "b2", "w1g", "wb1"):
        vec_d[v] = nc.dram_tensor(v, [P, MT], f32, kind="ExternalInput").ap()
    onesr_d = nc.dram_tensor("onesr", [P, P], f32r, kind="ExternalInput").ap()
    onesb_d = nc.dram_tensor("onesb", [P, P], bf16, kind="ExternalInput").ap()
    out_d = nc.dram_tensor("out", [MT, P, TQ], f32, kind="ExternalOutput").ap()

    with tile.TileContext(nc) as tc, \
         nc.allow_low_precision(reason="bf16 pipeline, f32 psum accumulation"):
        with tc.tile_pool(name="persist", bufs=1) as pp, \
             tc.tile_pool(name="ptile", bufs=7) as ppool, \
             tc.tile_pool(name="small", bufs=6) as sp, \
             tc.tile_pool(name="sq", bufs=4) as sqp, \
             tc.tile_pool(name="tmp", bufs=4) as tp, \
             tc.tile_pool(name="outp", bufs=2) as op_pool, \
             tc.tile_pool(name="ps_s", bufs=5, space="PSUM") as ps_s, \
             tc.tile_pool(name="ps_o", bufs=2, space="PSUM") as ps_o, \
             tc.tile_pool(name="ps_w", bufs=1, space="PSUM") as ps_w:

            # ---- persistent tiles ----
            xq_sb = pp.tile([P, MT, TQ], bf16, tag="xq")
            xqf_sb = pp.tile([P, MT, TQ], f32r, tag="xqf")
            xk_sb = pp.tile([P, MT, nkvv], bf16, tag="xk", name="xk_sb")
            xv_sb = pp.tile([P, MT, nkvv], bf16, tag="xv", name="xv_sb")
            wsb = {w: pp.tile([P, MT, MT, P], bf16, tag=f"w_{w}",
                              name=f"wsb_{w}")
                   for w in ("wk", "wq", "wp", "w1")}
            wv_sb = pp.tile([P, MT, DIM], bf16, tag="wv")
            k_lo = pp.tile([P, 4, njv, P], bf16, tag="klo")
            k_hi = pp.tile([P, 4, njv, P], bf16, tag="khi")
            q_sb = pp.tile([P, MT, TQ], bf16, tag="q", name="q_sb")
            v_sb = pp.tile([P, njv, HEADS, HD + 1], bf16, tag="v", name="v_sb")
            x1 = pp.tile([P, MT, TQ], bf16, tag="x1")
            z_sb = pp.tile([P, MT, TQ], bf16, tag="xq", name="z_sb")
            maskb = pp.tile([P, njv], f32, tag="mask")
            ones128 = pp.tile([P, 1], bf16, tag="ones128")
            ones128r = pp.tile([P, 1], f32r, tag="ones128r")
            ones1r = pp.tile([1, P], f32r, tag="ones1r")
            vec = {v: pp.tile([P, MT], f32, tag=f"vec_{v}", name=f"sb_{v}")
                   for v in vec_d}

            # ---- tiny init: constants via DMA (memset fails ISA checks) ----
            nc.gpsimd.dma_start(ones128[:], onesb_d[:, 0:1])
            nc.gpsimd.dma_start(ones128r[:], onesr_d[:, 0:1])
            nc.gpsimd.dma_start(ones1r[:], onesr_d[0:1, :])
            nc.gpsimd.dma_start(
                v_sb[:, :, :, HD:HD + 1],
                onesb_d[:, 0:njv * HEADS].rearrange(
                    "p (j h) -> p j h", j=njv).unsqueeze(-1))
            nc.gpsimd.dma_start(maskb[:], mask_d)
            for v in vec_d:
                nc.gpsimd.dma_start(vec[v][:], vec_d[v])

            # ---- input/weight DMAs ----
            # SP queue: activations first, then wp/w1/w2 (needed late).
            HTQ = TQ // 2
            for c in range(2):
                nc.sync.dma_start(xq_sb[:, :, c * HTQ:(c + 1) * HTQ],
                                  xqT_d[:, :, c * HTQ:(c + 1) * HTQ])
            kchunks = [(0, 512), (512, nkvv)] if nkvv > 512 else [(0, nkvv)]
            for c0, c1 in kchunks:
                nc.sync.dma_start(xk_sb[:, :, c0:c1], xkT_d[:, :, c0:c1])
            for c0, c1 in kchunks:
                nc.sync.dma_start(xv_sb[:, :, c0:c1], xvT_d[:, :, c0:c1])
            nc.sync.dma_start(xqf_sb[:], xqTf_d)
            for w in ("wp", "w1"):
                for mt in range(MT):
                    nc.sync.dma_start(wsb[w][:, mt], wt_d[w][mt])
            # Act queue: wq/wk/wv (Act engine idle until attention).
            for w in ("wq", "wk"):
                for mt in range(MT):
                    nc.scalar.dma_start(wsb[w][:, mt], wt_d[w][mt])
            for h in range(2):
                nc.scalar.dma_start(wv_sb[:, :, h * 512:(h + 1) * 512],
                                    wv_d[:, :, h * 512:(h + 1) * 512])

            copy_eng = [nc.vector, nc.vector]

            # ---- Q projection (2 column passes for early start) ----
            for c in range(2):
                for mt in range(MT):
                    ps = ps_s.tile([P, TQ], f32, tag="s")
                    for kt in range(MT):
                        nc.tensor.matmul(
                            ps[:, 0:HTQ], wsb["wq"][:, mt, kt, :],
                            xq_sb[:, kt, c * HTQ:(c + 1) * HTQ],
                            start=(kt == 0), stop=(kt == MT - 1))
                    copy_eng[mt % 2].tensor_copy(
                        q_sb[:, mt, c * HTQ:(c + 1) * HTQ], ps[:, 0:HTQ])

            # ---- K projection: K^T tiles -> k_lo/k_hi (bf16) ----
            for mt in range(MT):
                kdst = k_lo if mt < 4 else k_hi
                for c0, c1 in kchunks:
                    cw = c1 - c0
                    jn = cw // P
                    ps = ps_s.tile([P, TQ], f32, tag="s")
                    for kt in range(MT):
                        nc.tensor.matmul(
                            ps[:, 0:cw], wsb["wk"][:, mt, kt, :],
                            xk_sb[:, kt, c0:c1],
                            start=(kt == 0), stop=(kt == MT - 1))
                    if mt % 2:
                        nc.scalar.copy(
                            kdst[:, mt % 4, c0 // P:c0 // P + jn, :],
                            ps[:, 0:cw].rearrange("p (j c) -> p j c", j=jn))
                    else:
                        nc.vector.tensor_copy(
                            kdst[:, mt % 4, c0 // P:c0 // P + jn, :],
                            ps[:, 0:cw].rearrange("p (j c) -> p j c", j=jn))

            # ---- V projection unit (swapped roles: out partitions = kv) ----
            def v_unit(j, half):
                ps = ps_o.tile([P, TQ], f32, tag="o", name=f"v{j}_{half}")
                for kt in range(MT):
                    nc.tensor.matmul(
                        ps[:], xv_sb[:, kt, j * P:(j + 1) * P],
                        wv_sb[:, kt, half * 512:(half + 1) * 512],
                        start=(kt == 0), stop=(kt == MT - 1))
                nc.vector.tensor_copy(
                    v_sb[:, j, 8 * half:8 * half + 8, 0:HD],
                    ps[:].rearrange("p (h d) -> p h d", h=8))

            def v_half_unit(j, half, ch):
                # quarter of a v j-tile: 4 heads' worth of columns
                ps = ps_o.tile([P, TQ], f32, tag="o", name=f"v{j}_{half}_{ch}")
                c0 = ch * 256
                for kt in range(MT):
                    nc.tensor.matmul(
                        ps[:, 0:256], xv_sb[:, kt, j * P:(j + 1) * P],
                        wv_sb[:, kt, half * 512 + c0:half * 512 + c0 + 256],
                        start=(kt == 0), stop=(kt == MT - 1))
                nc.vector.tensor_copy(
                    v_sb[:, j, 8 * half + 4 * ch:8 * half + 4 * ch + 4, 0:HD],
                    ps[:, 0:256].rearrange("p (h d) -> p h d", h=4))

            INTERLEAVE = True
            # half-0 heads (0-7) need v columns 0:64 of each j before o-mms
            for j in range(njv):
                v_unit(j, 0)
            if INTERLEAVE is False:
                for j in range(njv):
                    v_unit(j, 1)

            # w2 reuses wv's SBUF (wv dead after V proj); loads during attn
            w2_sb = pp.tile([P, MT, MT, P], bf16, tag="wv", name="w2_sb")

            # ---- attention, with V half-1 and Wp columns 0-2 interleaved ----
            o_sb = pp.tile([P, MT, TQ], bf16, tag="xk", name="o_sb")
            xres = pp.tile([P, MT, TQ], f32r, tag="q", name="xres")
            psw = [ps_w.tile([P, TQ], f32, tag="w", name=f"wpA{m}")
                   for m in range(1)]

            def wp_passA(kt):
                for m in range(1):
                    nc.tensor.matmul(psw[m][:], wsb["wp"][:, m, kt, :],
                                     o_sb[:, kt, :],
                                     start=(kt == 0), stop=(kt == MT - 1))

            def s_mm(h, j, s_tiles):
                lo = 64 * (h % 2)
                ktile = k_lo if h < 8 else k_hi
                mtl = (h // 2) % 4
                s_ps = ps_s.tile([P, TQ], f32, tag="s", name=f"s{h}_{j}")
                nc.tensor.matmul(
                    s_ps[:], ktile[lo:lo + 64, mtl, j, :],
                    q_sb[lo:lo + 64, h // 2, :], start=True, stop=True)
                s_tiles[(h, j)] = s_ps

            s_tiles = {}
            for j in range(njv):
                s_mm(0, j, s_tiles)
            for h in range(HEADS):
                lo = 64 * (h % 2)
                p_tiles = []
                for j in range(njv):
                    p_t = ppool.tile([P, TQ], bf16, tag="p", name=f"p{h}_{j}")
                    nc.scalar.activation(p_t[:], s_tiles.pop((h, j))[:],
                                         AF.Exp, bias=maskb[:, j:j + 1],
                                         scale=SCALE)
                    p_tiles.append(p_t)
                if h < HEADS - 1:
                    s_mm(h + 1, 0, s_tiles)
                    s_mm(h + 1, 1, s_tiles)
                if INTERLEAVE and h < 2 * njv:
                    v_half_unit(h % njv, 1, h // njv)  # PE fills during exps
                o_ps = ps_o.tile([P, TQ], f32, tag="o", name=f"o{h}")
                for j in range(njv):
                    if h < HEADS - 1 and j >= 2:
                        s_mm(h + 1, j, s_tiles)
                    nc.tensor.matmul(o_ps[0:HD + 1, :], v_sb[:, j, h, :],
                                     p_tiles[j][:],
                                     start=(j == 0), stop=(j == njv - 1))
                srow = sp.tile([1, TQ], f32r, tag="srow", name=f"sr{h}")
                nc.vector.reciprocal(srow[0:1, :], o_ps[HD:HD + 1, :])
                if INTERLEAVE is True and h % 2 == 1 and h >= 3:
                    wp_passA((h - 3) // 2)  # hides the reciprocal latency
                b_ps = ps_o.tile([P, TQ], f32, tag="o", name=f"b{h}")
                nc.tensor.matmul(b_ps[:], ones1r[:], srow[:],
                                 start=True, stop=True)
                dst = o_sb[lo:lo + 64, h // 2, :]
                nc.vector.tensor_copy(dst, o_ps[0:HD, :])
                nc.vector.tensor_tensor(dst, dst, b_ps[lo:lo + 64, :],
                                        OP.mult)
                if h == 5:
                    for mt in range(MT):
                        nc.sync.dma_start(w2_sb[:, mt], wt_d["w2"][mt])
            if INTERLEAVE is True:
                wp_passA(7)
            else:
                for kt in range(MT):
                    wp_passA(kt)
            # preload sqrt act-table while Act is idle (exp set -> sqrt set)
            warm = sp.tile([1, TQ], f32r, tag="srow", name="warm_sqrt")
            nc.scalar.sqrt(warm[0:1, 0:1], ones1r[0:1, 0:1])

            # ---- Wp epilogues + pass B, with LN1 sums interleaved ----
            sum1_ps = ps_s.tile([1, TQ], f32, tag="s", name="lnsum_g1")
            sq1_ps = ps_s.tile([1, TQ], f32, tag="s", name="lnsq_g1")

            def ln_accum(src, mt, sum_ps, sq_ps):
                sq = sqp.tile([P, TQ], bf16, tag="sq", bufs=2)
                nc.gpsimd.tensor_tensor(sq[:], src[:, mt, :], src[:, mt, :],
                                        OP.mult)
                nc.tensor.matmul(sum_ps[:], ones128r[:], src[:, mt, :],
                                 start=(mt == 0), stop=(mt == MT - 1))
                nc.tensor.matmul(sq_ps[:], ones128[:], sq[:],
                                 start=(mt == 0), stop=(mt == MT - 1))

            psu = [ps_s.tile([P, TQ], f32, tag="s", name=f"uA{m}")
                   for m in range(2)]

            def wp_epi(m, ps):
                nc.vector.scalar_tensor_tensor(
                    xres[:, m, :], ps[:], vec["bp"][:, m:m + 1],
                    xqf_sb[:, m, :], OP.add, OP.add)
                # z = g1*xres feeds FFN1 (U = W1 z) with no LN-stats dep
                nc.gpsimd.tensor_scalar_mul(z_sb[:, m, :], xres[:, m, :],
                                            vec["g1"][:, m:m + 1])
                for mo in range(2):
                    nc.tensor.matmul(psu[mo][:], wsb["w1"][:, mo, m, :],
                                     z_sb[:, m, :],
                                     start=(m == 0), stop=(m == MT - 1))

            for m in range(1):
                wp_epi(m, psw[m])
            for m in range(1, MT):
                if m >= 2:
                    ln_accum(xres, m - 2, sum1_ps, sq1_ps)
                ps = (ps_w.tile([P, TQ], f32, tag="w", name=f"wpB{m}")
                      if m % 2 else
                      ps_o.tile([P, TQ], f32, tag="o", name=f"wpB{m}"))
                for kt in range(MT):
                    nc.tensor.matmul(ps[:], wsb["wp"][:, m, kt, :],
                                     o_sb[:, kt, :],
                                     start=(kt == 0), stop=(kt == MT - 1))
                wp_epi(m, ps)
            ln_accum(xres, 6, sum1_ps, sq1_ps)
            ln_accum(xres, 7, sum1_ps, sq1_ps)

            def ln_stats(sum_ps, sq_ps, gname, c0, cw):
                """Stats chain on columns [c0:c0+cw]; returns (a_ps, b_ps)."""
                cs = slice(c0, c0 + cw)
                mean = sp.tile([1, TQ], f32r, tag="srow", name=f"mean_{gname}")
                nc.vector.tensor_scalar_mul(mean[0:1, cs], sum_ps[0:1, cs],
                                            1.0 / DIM)
                nmsq = sp.tile([1, TQ], f32r, tag="srow", name=f"nmsq_{gname}")
                nc.gpsimd.tensor_tensor(nmsq[0:1, cs], mean[0:1, cs],
                                        mean[0:1, cs], OP.mult)
                var = sp.tile([1, TQ], f32r, tag="srow", name=f"var_{gname}")
                nc.vector.scalar_tensor_tensor(var[0:1, cs], sq_ps[0:1, cs],
                                               1.0 / DIM, nmsq[0:1, cs],
                                               OP.mult, OP.subtract)
                rvar = sp.tile([1, TQ], f32r, tag="srow", name=f"rvar_{gname}")
                nc.vector.reciprocal(rvar[0:1, cs], var[0:1, cs])
                rstd = sp.tile([1, TQ], f32r, tag="srow", name=f"rstd_{gname}")
                nc.scalar.sqrt(rstd[0:1, cs], rvar[0:1, cs])
                nmr = sp.tile([1, TQ], f32r, tag="srow", name=f"nmr_{gname}")
                nc.vector.scalar_tensor_tensor(nmr[0:1, cs], mean[0:1, cs],
                                               -1.0, rstd[0:1, cs],
                                               OP.mult, OP.mult)
                return rstd, nmr

            def make_reps(rstd, nmr, gname, c0, cw, pair=None):
                """Replicate rstd/nmr rows across partitions -> SBUF f32r.
                Pass `pair` to write another column slice of the same tiles
                (avoids buffer-rotation serialization between halves)."""
                cs = slice(c0, c0 + cw)
                a_ps = ps_s.tile([P, TQ], f32, tag="s", name=f"ar_{gname}")
                nc.tensor.matmul(a_ps[:, cs], ones1r[:], rstd[0:1, cs],
                                 start=True, stop=True)
                b_ps = ps_o.tile([P, TQ], f32, tag="o", name=f"br_{gname}")
                nc.tensor.matmul(b_ps[:, cs], ones1r[:], nmr[0:1, cs],
                                 start=True, stop=True)
                if pair is None:
                    a_bf = sqp.tile([P, TQ], f32r, tag="sqr", bufs=2,
                                    name=f"abf_{gname}")
                    b_bf = sqp.tile([P, TQ], f32r, tag="sqr", bufs=2,
                                    name=f"bbf_{gname}")
                else:
                    a_bf, b_bf = pair
                nc.vector.tensor_copy(a_bf[:, cs], a_ps[:, cs])
                nc.scalar.copy(b_bf[:, cs], b_ps[:, cs])
                return a_bf, b_bf

            def ln_norm(src, a_bf, b_bf, gname, bname, t_dtype, write_fn,
                        mt, c0, cw):
                cs = slice(c0, c0 + cw)
                t = tp.tile([P, TQ], t_dtype, tag="t")
                nc.vector.tensor_tensor(t[:, cs], src[:, mt, cs], a_bf[:, cs],
                                        OP.mult)
                nc.gpsimd.tensor_tensor(t[:, cs], t[:, cs], b_bf[:, cs],
                                        OP.add)
                write_fn(mt, cs, t)

            # ---- FFN1 from z (stats-independent), LN1 x1 for residual ----
            hf = pp.tile([P, MT, TQ], bf16, tag="xv", name="hf")

            def u_pass(m, tag):
                ps = ps_s.tile([P, TQ], f32, tag="s", name=f"uB{m}")
                for kt in range(MT):
                    nc.tensor.matmul(ps[:], wsb["w1"][:, m, kt, :],
                                     z_sb[:, kt, :],
                                     start=(kt == 0), stop=(kt == MT - 1))
                return ps

            def f1_epi(m, ps):
                # hf = relu(rstd*U + nmr*(W1 g1) + (W1 b1 + bf1))
                t = tp.tile([P, TQ], bf16, tag="t")
                nc.vector.tensor_tensor(t[:], ps[:], a1_bf[:], OP.mult)
                nc.vector.scalar_tensor_tensor(
                    t[:], b1_bf[:], vec["w1g"][:, m:m + 1], t[:],
                    OP.mult, OP.add)
                nc.scalar.activation(hf[:, m, :], t[:], AF.Relu,
                                     bias=vec["wb1"][:, m:m + 1], scale=1.0)

            ub = {m: u_pass(m, "w") for m in (2, 3)}  # runs during stats
            rstd1, nmr1 = ln_stats(sum1_ps, sq1_ps, "g1", 0, TQ)
            a1_bf, b1_bf = make_reps(rstd1, nmr1, "g1", 0, TQ)
            f1_epi(0, psu[0])
            f1_epi(1, psu[1])
            ub[4] = u_pass(4, "s")
            ub[5] = u_pass(5, "s")
            f1_epi(2, ub[2])
            f1_epi(3, ub[3])
            ub[6] = u_pass(6, "w")
            ub[7] = u_pass(7, "w")
            for m in (4, 5, 6, 7):
                f1_epi(m, ub[m])

            # x1 = (xres - mu)*rstd*g1 + b1 (residual only; on Pool/Act)
            for mt in range(MT):
                u = tp.tile([P, TQ], bf16, tag="t")
                nc.gpsimd.tensor_tensor(u[:], xres[:, mt, :], a1_bf[:],
                                        OP.mult)
                nc.gpsimd.tensor_tensor(u[:], u[:], b1_bf[:], OP.add)
                nc.scalar.activation(x1[:, mt, :], u[:], AF.Identity,
                                     bias=vec["b1"][:, mt:mt + 1],
                                     scale=vec["g1"][:, mt:mt + 1])

            # ---- FFN2 + bias + residual -> yres, LN2 sums interleaved ----
            yres = pp.tile([P, MT, TQ], f32r, tag="v", name="yres")
            sum2_ps = ps_s.tile([1, TQ], f32, tag="s", name="lnsum_g2")
            sq2_ps = ps_s.tile([1, TQ], f32, tag="s", name="lnsq_g2")
            for mt in range(MT):
                if mt >= 1:
                    ln_accum(yres, mt - 1, sum2_ps, sq2_ps)
                ps = (ps_w.tile([P, TQ], f32, tag="w", name=f"f2_{mt}")
                      if mt % 2 else
                      ps_o.tile([P, TQ], f32, tag="o", name=f"f2_{mt}"))
                for kt in range(MT):
                    nc.tensor.matmul(ps[:], w2_sb[:, mt, kt, :],
                                     hf[:, kt, :],
                                     start=(kt == 0), stop=(kt == MT - 1))
                nc.vector.scalar_tensor_tensor(
                    yres[:, mt, :], ps[:], vec["bf2"][:, mt:mt + 1],
                    x1[:, mt, :], OP.add, OP.add)
            ln_accum(yres, 7, sum2_ps, sq2_ps)

            # ---- LN2 -> DRAM (f32), column-split tail ----
            # A(mt) = g2 x rstd and B(mt) = b2 x 1 + g2 x (-mu*rstd) as PE
            # outer products: normalize is then 2 tensor ops per tile.
            rows2 = {c0: ln_stats(sum2_ps, sq2_ps, f"g2c{c0}", c0, HTQ)
                     for c0 in (0, HTQ)}
            pair2 = None
            for c0 in (0, HTQ):
                pair2 = make_reps(*rows2[c0], f"g2c{c0}", c0, HTQ, pair2)
            reps2 = {0: pair2, HTQ: pair2}
            for c0 in (0, HTQ):
                a_bf, b_bf = reps2[c0]
                cs = slice(c0, c0 + HTQ)
                for mt in range(MT):
                    t = tp.tile([P, HTQ], f32r, tag="t")
                    nc.gpsimd.tensor_tensor(t[:], yres[:, mt, cs],
                                            a_bf[:, cs], OP.mult)
                    nc.vector.tensor_tensor(t[:], t[:], b_bf[:, cs], OP.add)
                    ot = op_pool.tile([P, HTQ], f32, tag="out", bufs=3,
                                      name=f"out{mt}_{c0}")
                    nc.scalar.activation(ot[:], t[:], AF.Identity,
                                         bias=vec["b2"][:, mt:mt + 1],
                                         scale=vec["g2"][:, mt:mt + 1])
                    (nc.sync if mt % 2 == 0 else nc.gpsimd).dma_start(
                        out_d[mt][:, cs], ot[:])

    nc.compile()
    return nc


def _np_bf16():
    import concourse.mybir as mybir
    return mybir.dt.np(mybir.dt.bfloat16)


def _host_prep(inputs):
    """Gather valid kv columns, cast to bf16, tile-transpose."""
    bf16 = _np_bf16()
    mask = np.asarray(inputs["mask"])
    njv = int(max(int(np.ceil(int(mask[b].sum()) / P)) for b in range(B)))
    njv = max(njv, 1)
    nkvv = njv * P

    def xt(x):  # [n, DIM] f32 -> [P, MT, n] bf16
        return np.ascontiguousarray(
            x.T.reshape(MT, P, x.shape[0]).transpose(1, 0, 2)).astype(bf16)

    def wtiles(w):
        wt = w.T  # [k, m]
        return np.ascontiguousarray(
            wt.reshape(MT, P, MT, P).transpose(2, 1, 0, 3)).astype(bf16)

    def vecp(v):
        return np.ascontiguousarray(v.reshape(MT, P).T).astype(np.float32)

    xkT, xvT, maskbs = [], [], []
    for b in range(B):
        ib = np.nonzero(mask[b])[0]
        pad = nkvv - len(ib)
        idx = np.concatenate([ib, np.zeros(pad, ib.dtype)])
        mb = np.concatenate([np.zeros(len(ib), np.float32),
                             np.full(pad, NEG, np.float32)])
        maskbs.append(np.ascontiguousarray(mb.reshape(njv, P).T))
        xkT.append(xt(np.asarray(inputs["x_k"])[b][idx]))
        xvT.append(xt(np.asarray(inputs["x_v"])[b][idx]))

    host = {
        "njv": njv,
        "xkT": xkT,
        "xvT": xvT,
        "maskb": maskbs,
        "shared": {
            "onesr": np.ones((P, P), np.float32),
            "onesb": np.ones((P, P), _np_bf16()),
            "wv_r": np.ascontiguousarray(
                np.asarray(inputs["Wv"]).T.reshape(MT, P, DIM)
                .transpose(1, 0, 2)).astype(bf16),
            "wk": wtiles(np.asarray(inputs["Wk"])),
            "wq": wtiles(np.asarray(inputs["Wq"])),
            "wp": wtiles(np.asarray(inputs["Wp"])),
            "w1": wtiles(np.asarray(inputs["W1"])),
            "w2": wtiles(np.asarray(inputs["W2"])),
            "bp": vecp(np.asarray(inputs["bp"])),
            "w1g": vecp(np.asarray(inputs["W1"]).astype(np.float64)
                        @ np.asarray(inputs["g_ln1"]).astype(np.float64)),
            "wb1": vecp((np.asarray(inputs["W1"]).astype(np.float64)
                         @ np.asarray(inputs["b_ln1"]).astype(np.float64))
                        + np.asarray(inputs["bf1"]).astype(np.float64)),
            "bf1": vecp(np.asarray(inputs["bf1"])),
            "bf2": vecp(np.asarray(inputs["bf2"])),
            "g1": vecp(np.asarray(inputs["g_ln1"])),
            "b1": vecp(np.asarray(inputs["b_ln1"])),
            "g2": vecp(np.asarray(inputs["g_ln2"])),
            "b2": vecp(np.asarray(inputs["b_ln2"])),
        },
    }
    return host


def _prep_core(inputs, b, qh, host):
    bf16 = _np_bf16()
    xq = np.asarray(inputs["x_q"])[b, qh * TQ:(qh + 1) * TQ, :]
    xqt = np.ascontiguousarray(xq.T.reshape(MT, P, TQ).transpose(1, 0, 2))
    d = {
        "xqT": xqt.astype(bf16),
        "xqTf": xqt.astype(np.float32),
        "xkT": host["xkT"][b],
        "xvT": host["xvT"][b],
        "maskb": host["maskb"][b],
    }
    d.update(host["shared"])
    return d


def get_nc(njv):
    key = ("nc", njv)
    if key not in _CACHE:
        _CACHE[key] = _build(njv)
    return _CACHE[key]


def kernel(**inputs):
    from concourse.bass_utils import run_bass_kernel_spmd
    inputs = {k: np.asarray(v) for k, v in inputs.items()}
    host = _host_prep(inputs)
    nc = get_nc(host["njv"])
    in_maps = []
    for c in range(8):
        in_maps.append(_prep_core(inputs, c // 2, c % 2, host))
    res = run_bass_kernel_spmd(nc, in_maps, list(range(8)))
    out = np.empty((B, NQ, DIM), np.float32)
    for c in range(8):
        b, qh = c // 2, c % 2
        oc = np.asarray(res.results[c]["out"]).astype(np.float32)  # [mt, p, q]
        out[b, qh * TQ:(qh + 1) * TQ, :] = (
            oc.transpose(2, 0, 1).reshape(TQ, DIM))
    return out



# revision 34
# speedup vs baseline: 1.0346x; 1.0346x over previous
"""Trainium2 Bass kernel for a dense transformer block.

Model (B=4, N=1024, D=1024, H=16, hd=64):
  q/k/v = x{q,k,v} @ W{q,k,v}.T ; attn = softmax(mask(q k^T / 8)) @ v
  x1 = LN1(x_q + attn_out @ Wp.T + bp)
  out = LN2(x1 + relu(x1 @ W1.T + bf1) @ W2.T + bf2)

Sharding: 8 cores = (batch b, query-half qh); each core owns 512 queries.

HW-trace-driven tuning (NTFF profile; measured exec 367us -> 259us):
- GpSimd TENSOR_SCALAR is ~7.5us/op on HW (vs ~0.5us on DVE): z = g1*xres
  moved to DVE.
- DVE builtin RECIPROCAL is iterative (~3.2us/row): softmax denominators
  and LN 1/var use reciprocal_approx_fast (~5x faster, 18-bit). The
  custom-DVE op needs its input at partition 0, so the denom row (PSUM
  partition 64) is staged through SBUF with a builtin copy first.
- softmax 1/den rows broadcast via bf16 PE matmul (full rate) instead of
  f32r (1/4 rate).
- LN1 sums come from the existing bf16 z tiles: sum(xres) = (1/g1)^T z,
  sum(xres^2) = (1/g1^2)^T z^2 (host precomputes 1/g1, 1/g1^2; |g1| must
  be > ~1e-30, true for any sane LN scale) — replaces 1/4-rate fp32 PE
  reductions.
- Q projection is one 512-wide pass (half the PE instructions; per-matmul
  fixed overhead ~200-300ns dominates small matmuls on HW).

Design (sim: 163us vs 265us for the f32r baseline):
- KV gather: only mask-valid kv columns are shipped/computed (padded to a
  multiple of 128; padding gets a -1e5 score bias so exp()==0 exactly).
  Shrinks K/V projection, scores, softmax, and attn-out by ~3/8 (njv=5).
  The program is compiled per njv (cached); inputs are deterministic.
- bf16 matmul operands (f32 PSUM accumulation): same PE rate as f32r at
  free>=256, half the DMA bytes and SBUF, 2x DVE. Residual/LN inputs
  (xqf, xres, yres) stay f32 to keep max-norm rel err ~4.4e-3 (<2e-2).
- Feature-major layout: activations live as x^T[d, n] with d = mt*128 + p.
  Partition-axis reductions (softmax denom, LN sums) go through the PE via
  ones-vector matmuls / an appended ones column in V.
- Software pipelining: V-half1 projection (quarter units) and 1 column of
  Wp accumulate inside the attention loop (fills PE while Act runs exps);
  5 score-PSUM buffers give the exp pipeline a full head of slack;
  LN sums ride behind the Wp/FFN2 epilogues; FFN1 runs on z = g1*xres with
  the LN1 correction applied algebraically afterwards (W1 x1 = rstd*(W1 z)
  + nmr*(W1 g1) + W1 b1), so the PE never waits on LN1 stats; LN2 runs
  column-split with a pipelined DVE/Pool/Act/2-queue-DMA tail.
- HW legality (BIR verifier): GPSIMD/Pool never touches PSUM (copies and
  LN middles on DVE/Act instead), one-PSUM-operand rule respected, no
  memsets (constants DMA'd from a ones input), rstd = Act-Sqrt of DVE
  reciprocal; sqrt act-table preloaded after attention (2 loads total).
- Weights stream per-128-row-tile on the SP and Act DMA queues; w2 shares
  wv's SBUF (loads during attention); out DMAs overlap the LN2 tail.
"""
import numpy as np

P = 128
DIM = 1024
HEADS = 16
HD = 64
B = 4
NQ = 1024
NKV = 1024
TQ = 512          # queries per core
MT = DIM // P     # 8 feature tiles
SCALE = HD ** -0.5
NEG = -1e5

_CACHE = {}


def _build(njv):
    import concourse.bass as bass
    import concourse.mybir as mybir
    import concourse.tile as tile
    from concourse import bacc

    f32 = mybir.dt.float32
    f32r = mybir.dt.float32r
    bf16 = mybir.dt.bfloat16
    AF = mybir.ActivationFunctionType
    OP = mybir.AluOpType

    nkvv = njv * P

    nc = bacc.Bacc("TRN2", target_bir_lowering=False, debug=False)

    xqT_d = nc.dram_tensor("xqT", [P, MT, TQ], bf16, kind="ExternalInput").ap()
    xqTf_d = nc.dram_tensor("xqTf", [P, MT, TQ], f32r, kind="ExternalInput").ap()
    xkT_d = nc.dram_tensor("xkT", [P, MT, nkvv], bf16, kind="ExternalInput").ap()
    xvT_d = nc.dram_tensor("xvT", [P, MT, nkvv], bf16, kind="ExternalInput").ap()
    wv_d = nc.dram_tensor("wv_r", [P, MT, DIM], bf16, kind="ExternalInput").ap()
    wt_d = {}
    for w in ("wk", "wq", "wp", "w1", "w2"):
        wt_d[w] = nc.dram_tensor(w, [MT, P, MT, P], bf16, kind="ExternalInput").ap()
    mask_d = nc.dram_tensor("maskb", [P, njv], f32, kind="ExternalInput").ap()
    vec_d = {}
    for v in ("bp", "bf1", "bf2", "g1", "b1", "g2", "b2", "w1g", "wb1"):
        vec_d[v] = nc.dram_tensor(v, [P, MT], f32, kind="ExternalInput").ap()
    bvec_d = {}
    for v in ("rg1b", "rg1sb"):
        bvec_d[v] = nc.dram_tensor(v, [P, MT], bf16, kind="ExternalInput").ap()
    onesr_d = nc.dram_tensor("onesr", [P, P], f32r, kind="ExternalInput").ap()
    onesb_d = nc.dram_tensor("onesb", [P, P], bf16, kind="ExternalInput").ap()
    out_d = nc.dram_tensor("out", [MT, P, TQ], f32, kind="ExternalOutput").ap()

    with tile.TileContext(nc) as tc, \
         nc.allow_low_precision(reason="bf16 pipeline, f32 psum accumulation"):
        with tc.tile_pool(name="persist", bufs=1) as pp, \
             tc.tile_pool(name="ptile", bufs=7) as ppool, \
             tc.tile_pool(name="small", bufs=5) as sp, \
             tc.tile_pool(name="sq", bufs=4) as sqp, \
             tc.tile_pool(name="tmp", bufs=3) as tp, \
             tc.tile_pool(name="outp", bufs=2) as op_pool, \
             tc.tile_pool(name="ps_s", bufs=5, space="PSUM") as ps_s, \
             tc.tile_pool(name="ps_o", bufs=2, space="PSUM") as ps_o, \
             tc.tile_pool(name="ps_w", bufs=1, space="PSUM") as ps_w:

            # ---- persistent tiles ----
            xq_sb = pp.tile([P, MT, TQ], bf16, tag="xq")
            xqf_sb = pp.tile([P, MT, TQ], f32r, tag="xqf")
            xk_sb = pp.tile([P, MT, nkvv], bf16, tag="xk", name="xk_sb")
            xv_sb = pp.tile([P, MT, nkvv], bf16, tag="xv", name="xv_sb")
            wsb = {w: pp.tile([P, MT, MT, P], bf16, tag=f"w_{w}",
                              name=f"wsb_{w}")
                   for w in ("wk", "wq", "wp", "w1")}
            wv_sb = pp.tile([P, MT, DIM], bf16, tag="wv")
            k_lo = pp.tile([P, 4, njv, P], bf16, tag="klo")
            k_hi = pp.tile([P, 4, njv, P], bf16, tag="khi")
            q_sb = pp.tile([P, MT, TQ], bf16, tag="q", name="q_sb")
            v_sb = pp.tile([P, njv, HEADS, HD + 1], bf16, tag="v", name="v_sb")
            x1 = pp.tile([P, MT, TQ], bf16, tag="x1")
            z_sb = pp.tile([P, MT, TQ], bf16, tag="xq", name="z_sb")
            maskb = pp.tile([P, njv], f32, tag="mask")
            ones128 = pp.tile([P, 1], bf16, tag="ones128")
            ones128r = pp.tile([P, 1], f32r, tag="ones128r")
            ones1r = pp.tile([1, P], f32r, tag="ones1r")
            ones1b = pp.tile([1, P], bf16, tag="ones1b")
            vec = {v: pp.tile([P, MT], f32, tag=f"vec_{v}", name=f"sb_{v}")
                   for v in vec_d}
            bvec = {v: pp.tile([P, MT], bf16, tag=f"bvec_{v}", name=f"sbb_{v}")
                    for v in bvec_d}

            # ---- tiny init: constants via DMA (memset fails ISA checks) ----
            nc.gpsimd.dma_start(ones128[:], onesb_d[:, 0:1])
            nc.gpsimd.dma_start(ones128r[:], onesr_d[:, 0:1])
            nc.gpsimd.dma_start(ones1r[:], onesr_d[0:1, :])
            nc.gpsimd.dma_start(ones1b[:], onesb_d[0:1, :])
            nc.gpsimd.dma_start(
                v_sb[:, :, :, HD:HD + 1],
                onesb_d[:, 0:njv * HEADS].rearrange(
                    "p (j h) -> p j h", j=njv).unsqueeze(-1))
            nc.gpsimd.dma_start(maskb[:], mask_d)
            for v in vec_d:
                nc.gpsimd.dma_start(vec[v][:], vec_d[v])
            for v in bvec_d:
                nc.gpsimd.dma_start(bvec[v][:], bvec_d[v])

            # ---- input/weight DMAs ----
            # SP queue: activations first, then wp/w1/w2 (needed late).
            HTQ = TQ // 2
            for c in range(2):
                nc.sync.dma_start(xq_sb[:, :, c * HTQ:(c + 1) * HTQ],
                                  xqT_d[:, :, c * HTQ:(c + 1) * HTQ])
            kchunks = [(0, 512), (512, nkvv)] if nkvv > 512 else [(0, nkvv)]
            for c0, c1 in kchunks:
                nc.sync.dma_start(xk_sb[:, :, c0:c1], xkT_d[:, :, c0:c1])
            for c0, c1 in kchunks:
                nc.sync.dma_start(xv_sb[:, :, c0:c1], xvT_d[:, :, c0:c1])
            nc.sync.dma_start(xqf_sb[:], xqTf_d)
            for w in ("wp", "w1"):
                for mt in range(MT):
                    nc.sync.dma_start(wsb[w][:, mt], wt_d[w][mt])
            # Act queue: wq/wk/wv (Act engine idle until attention).
            for w in ("wq", "wk"):
                for mt in range(MT):
                    nc.scalar.dma_start(wsb[w][:, mt], wt_d[w][mt])
            for h in range(2):
                nc.scalar.dma_start(wv_sb[:, :, h * 512:(h + 1) * 512],
                                    wv_d[:, :, h * 512:(h + 1) * 512])

            copy_eng = [nc.vector, nc.vector]

            # ---- Q projection (single 512-wide pass: fewer PE instrs) ----
            for mt in range(MT):
                ps = ps_s.tile([P, TQ], f32, tag="s")
                for kt in range(MT):
                    nc.tensor.matmul(
                        ps[:], wsb["wq"][:, mt, kt, :], xq_sb[:, kt, :],
                        start=(kt == 0), stop=(kt == MT - 1))
                copy_eng[mt % 2].tensor_copy(q_sb[:, mt, :], ps[:])

            # ---- K projection: K^T tiles -> k_lo/k_hi (bf16) ----
            for mt in range(MT):
                kdst = k_lo if mt < 4 else k_hi
                for c0, c1 in kchunks:
                    cw = c1 - c0
                    jn = cw // P
                    ps = ps_s.tile([P, TQ], f32, tag="s")
                    for kt in range(MT):
                        nc.tensor.matmul(
                            ps[:, 0:cw], wsb["wk"][:, mt, kt, :],
                            xk_sb[:, kt, c0:c1],
                            start=(kt == 0), stop=(kt == MT - 1))
                    if mt % 2:
                        nc.scalar.copy(
                            kdst[:, mt % 4, c0 // P:c0 // P + jn, :],
                            ps[:, 0:cw].rearrange("p (j c) -> p j c", j=jn))
                    else:
                        nc.vector.tensor_copy(
                            kdst[:, mt % 4, c0 // P:c0 // P + jn, :],
                            ps[:, 0:cw].rearrange("p (j c) -> p j c", j=jn))

            # ---- V projection unit (swapped roles: out partitions = kv) ----
            def v_unit(j, half):
                ps = ps_o.tile([P, TQ], f32, tag="o", name=f"v{j}_{half}")
                for kt in range(MT):
                    nc.tensor.matmul(
                        ps[:], xv_sb[:, kt, j * P:(j + 1) * P],
                        wv_sb[:, kt, half * 512:(half + 1) * 512],
                        start=(kt == 0), stop=(kt == MT - 1))
                nc.vector.tensor_copy(
                    v_sb[:, j, 8 * half:8 * half + 8, 0:HD],
                    ps[:].rearrange("p (h d) -> p h d", h=8))

            def v_half_unit(j, half, ch):
                # quarter of a v j-tile: 4 heads' worth of columns
                ps = ps_o.tile([P, TQ], f32, tag="o", name=f"v{j}_{half}_{ch}")
                c0 = ch * 256
                for kt in range(MT):
                    nc.tensor.matmul(
                        ps[:, 0:256], xv_sb[:, kt, j * P:(j + 1) * P],
                        wv_sb[:, kt, half * 512 + c0:half * 512 + c0 + 256],
                        start=(kt == 0), stop=(kt == MT - 1))
                nc.vector.tensor_copy(
                    v_sb[:, j, 8 * half + 4 * ch:8 * half + 4 * ch + 4, 0:HD],
                    ps[:, 0:256].rearrange("p (h d) -> p h d", h=4))

            INTERLEAVE = True
            # half-0 heads (0-7) need v columns 0:64 of each j before o-mms
            for j in range(njv):
                v_unit(j, 0)
            if INTERLEAVE is False:
                for j in range(njv):
                    v_unit(j, 1)

            # w2 reuses wv's SBUF (wv dead after V proj); loads during attn
            w2_sb = pp.tile([P, MT, MT, P], bf16, tag="wv", name="w2_sb")

            # ---- attention, with V half-1 and Wp columns 0-2 interleaved ----
            o_sb = pp.tile([P, MT, TQ], bf16, tag="xk", name="o_sb")
            xres = pp.tile([P, MT, TQ], f32r, tag="q", name="xres")
            psw = [ps_w.tile([P, TQ], f32, tag="w", name=f"wpA{m}")
                   for m in range(1)]

            def wp_passA(kt):
                for m in range(1):
                    nc.tensor.matmul(psw[m][:], wsb["wp"][:, m, kt, :],
                                     o_sb[:, kt, :],
                                     start=(kt == 0), stop=(kt == MT - 1))

            def s_mm(h, j, s_tiles):
                lo = 64 * (h % 2)
                ktile = k_lo if h < 8 else k_hi
                mtl = (h // 2) % 4
                s_ps = ps_s.tile([P, TQ], f32, tag="s", name=f"s{h}_{j}")
                nc.tensor.matmul(
                    s_ps[:], ktile[lo:lo + 64, mtl, j, :],
                    q_sb[lo:lo + 64, h // 2, :], start=True, stop=True)
                s_tiles[(h, j)] = s_ps

            s_tiles = {}
            for j in range(njv):
                s_mm(0, j, s_tiles)
            for h in range(HEADS):
                lo = 64 * (h % 2)
                p_tiles = []
                for j in range(njv):
                    p_t = ppool.tile([P, TQ], bf16, tag="p", name=f"p{h}_{j}")
                    nc.scalar.activation(p_t[:], s_tiles.pop((h, j))[:],
                                         AF.Exp, bias=maskb[:, j:j + 1],
                                         scale=SCALE)
                    p_tiles.append(p_t)
                if h < HEADS - 1:
                    s_mm(h + 1, 0, s_tiles)
                    s_mm(h + 1, 1, s_tiles)
                if INTERLEAVE and h < 2 * njv:
                    v_half_unit(h % njv, 1, h // njv)  # PE fills during exps
                o_ps = ps_o.tile([P, TQ], f32, tag="o", name=f"o{h}")
                for j in range(njv):
                    if h < HEADS - 1 and j >= 2:
                        s_mm(h + 1, j, s_tiles)
                    nc.tensor.matmul(o_ps[0:HD + 1, :], v_sb[:, j, h, :],
                                     p_tiles[j][:],
                                     start=(j == 0), stop=(j == njv - 1))
                # custom-DVE recip needs its input at partition 0: stage the
                # denom row (PSUM partition 64) through SBUF with a builtin
                den = sp.tile([1, TQ], f32r, tag="srow", name=f"den{h}")
                nc.vector.tensor_copy(den[0:1, :], o_ps[HD:HD + 1, :])
                srow = sp.tile([1, TQ], f32, tag="srowf", bufs=1, name=f"sr{h}")
                nc.vector.reciprocal_approx_fast(
                    srow[0:1, :], den[0:1, :].bitcast(f32))
                srow_b = sp.tile([1, TQ], bf16, tag="srowb", bufs=2,
                                 name=f"srb{h}")
                nc.scalar.copy(srow_b[0:1, :], srow[0:1, :])
                if INTERLEAVE is True and h % 2 == 1 and h >= 3:
                    wp_passA((h - 3) // 2)  # hides the reciprocal latency
                b_ps = ps_o.tile([P, TQ], f32, tag="o", name=f"b{h}")
                nc.tensor.matmul(b_ps[:], ones1b[:], srow_b[:],
                                 start=True, stop=True)
                dst = o_sb[lo:lo + 64, h // 2, :]
                nc.vector.tensor_copy(dst, o_ps[0:HD, :])
                nc.vector.tensor_tensor(dst, dst, b_ps[lo:lo + 64, :],
                                        OP.mult)
                if h == 5:
                    for mt in range(MT):
                        nc.sync.dma_start(w2_sb[:, mt], wt_d["w2"][mt])
            if INTERLEAVE is True:
                wp_passA(7)
            else:
                for kt in range(MT):
                    wp_passA(kt)
            # preload sqrt act-table while Act is idle (exp set -> sqrt set)
            warm = sp.tile([1, TQ], f32r, tag="srow", name="warm_sqrt")
            nc.scalar.sqrt(warm[0:1, 0:1], ones1r[0:1, 0:1])

            # ---- Wp epilogues + pass B, with LN1 sums interleaved ----
            sum1_ps = ps_s.tile([1, TQ], f32, tag="s", name="lnsum_g1")
            sq1_ps = ps_s.tile([1, TQ], f32, tag="s", name="lnsq_g1")

            def ln_accum(src, mt, sum_ps, sq_ps):
                sq = sqp.tile([P, TQ], bf16, tag="sq", bufs=2)
                nc.gpsimd.tensor_tensor(sq[:], src[:, mt, :], src[:, mt, :],
                                        OP.mult)
                nc.tensor.matmul(sum_ps[:], ones128r[:], src[:, mt, :],
                                 start=(mt == 0), stop=(mt == MT - 1))
                nc.tensor.matmul(sq_ps[:], ones128[:], sq[:],
                                 start=(mt == 0), stop=(mt == MT - 1))

            def ln_accum_z(mt, sum_ps, sq_ps):
                # LN1 sums from z = g1*xres (bf16): sum(xres) = (1/g1)^T z,
                # sum(xres^2) = (1/g1^2)^T z^2 — full-rate bf16 PE reductions
                sq = sqp.tile([P, TQ], bf16, tag="sq", bufs=2)
                nc.gpsimd.tensor_tensor(sq[:], z_sb[:, mt, :], z_sb[:, mt, :],
                                        OP.mult)
                nc.tensor.matmul(sum_ps[:], bvec["rg1b"][:, mt:mt + 1],
                                 z_sb[:, mt, :],
                                 start=(mt == 0), stop=(mt == MT - 1))
                nc.tensor.matmul(sq_ps[:], bvec["rg1sb"][:, mt:mt + 1], sq[:],
                                 start=(mt == 0), stop=(mt == MT - 1))

            psu = [ps_s.tile([P, TQ], f32, tag="s", name=f"uA{m}")
                   for m in range(2)]

            def wp_epi(m, ps):
                nc.vector.scalar_tensor_tensor(
                    xres[:, m, :], ps[:], vec["bp"][:, m:m + 1],
                    xqf_sb[:, m, :], OP.add, OP.add)
                # z = g1*xres feeds FFN1 (U = W1 z) with no LN-stats dep
                # (DVE, not GpSimd: HW TENSOR_SCALAR on GpSimd is ~7.5us)
                nc.vector.tensor_scalar_mul(z_sb[:, m, :], xres[:, m, :],
                                            vec["g1"][:, m:m + 1])
                for mo in range(2):
                    nc.tensor.matmul(psu[mo][:], wsb["w1"][:, mo, m, :],
                                     z_sb[:, m, :],
                                     start=(m == 0), stop=(m == MT - 1))

            for m in range(1):
                wp_epi(m, psw[m])
            for m in range(1, MT):
                if m >= 2:
                    ln_accum_z(m - 2, sum1_ps, sq1_ps)
                ps = (ps_w.tile([P, TQ], f32, tag="w", name=f"wpB{m}")
                      if m % 2 else
                      ps_o.tile([P, TQ], f32, tag="o", name=f"wpB{m}"))
                for kt in range(MT):
                    nc.tensor.matmul(ps[:], wsb["wp"][:, m, kt, :],
                                     o_sb[:, kt, :],
                                     start=(kt == 0), stop=(kt == MT - 1))
                wp_epi(m, ps)
            ln_accum_z(6, sum1_ps, sq1_ps)
            ln_accum_z(7, sum1_ps, sq1_ps)

            def ln_stats(sum_ps, sq_ps, gname, c0, cw):
                """Stats chain on columns [c0:c0+cw]; returns (a_ps, b_ps)."""
                cs = slice(c0, c0 + cw)
                mean = sp.tile([1, TQ], f32r, tag="srow", name=f"mean_{gname}")
                nc.vector.tensor_scalar_mul(mean[0:1, cs], sum_ps[0:1, cs],
                                            1.0 / DIM)
                nmsq = sp.tile([1, TQ], f32r, tag="srow", name=f"nmsq_{gname}")
                nc.gpsimd.tensor_tensor(nmsq[0:1, cs], mean[0:1, cs],
                                        mean[0:1, cs], OP.mult)
                var = sp.tile([1, TQ], f32r, tag="srow", name=f"var_{gname}")
                nc.vector.scalar_tensor_tensor(var[0:1, cs], sq_ps[0:1, cs],
                                               1.0 / DIM, nmsq[0:1, cs],
                                               OP.mult, OP.subtract)
                rvar = sp.tile([1, TQ], f32, tag="srowf", bufs=1,
                               name=f"rvar_{gname}")
                nc.vector.reciprocal_approx_fast(
                    rvar[0:1, cs], var[0:1, cs].bitcast(f32))
                rstd = sp.tile([1, TQ], f32r, tag="srow", name=f"rstd_{gname}")
                nc.scalar.sqrt(rstd[0:1, cs], rvar[0:1, cs])
                nmr = sp.tile([1, TQ], f32r, tag="srow", name=f"nmr_{gname}")
                nc.vector.scalar_tensor_tensor(nmr[0:1, cs], mean[0:1, cs],
                                               -1.0, rstd[0:1, cs],
                                               OP.mult, OP.mult)
                return rstd, nmr

            def make_reps(rstd, nmr, gname, c0, cw, pair=None):
                """Replicate rstd/nmr rows across partitions -> SBUF f32r.
                Pass `pair` to write another column slice of the same tiles
                (avoids buffer-rotation serialization between halves)."""
                cs = slice(c0, c0 + cw)
                a_ps = ps_s.tile([P, TQ], f32, tag="s", name=f"ar_{gname}")
                nc.tensor.matmul(a_ps[:, cs], ones1r[:], rstd[0:1, cs],
                                 start=True, stop=True)
                b_ps = ps_o.tile([P, TQ], f32, tag="o", name=f"br_{gname}")
                nc.tensor.matmul(b_ps[:, cs], ones1r[:], nmr[0:1, cs],
                                 start=True, stop=True)
                if pair is None:
                    a_bf = sqp.tile([P, TQ], f32r, tag="sqr", bufs=2,
                                    name=f"abf_{gname}")
                    b_bf = sqp.tile([P, TQ], f32r, tag="sqr", bufs=2,
                                    name=f"bbf_{gname}")
                else:
                    a_bf, b_bf = pair
                nc.vector.tensor_copy(a_bf[:, cs], a_ps[:, cs])
                nc.scalar.copy(b_bf[:, cs], b_ps[:, cs])
                return a_bf, b_bf

            def ln_norm(src, a_bf, b_bf, gname, bname, t_dtype, write_fn,
                        mt, c0, cw):
                cs = slice(c0, c0 + cw)
                t = tp.tile([P, TQ], t_dtype, tag="t")
                nc.vector.tensor_tensor(t[:, cs], src[:, mt, cs], a_bf[:, cs],
                                        OP.mult)
                nc.gpsimd.tensor_tensor(t[:, cs], t[:, cs], b_bf[:, cs],
                                        OP.add)
                write_fn(mt, cs, t)

            # ---- FFN1 from z (stats-independent), LN1 x1 for residual ----
            hf = pp.tile([P, MT, TQ], bf16, tag="xv", name="hf")

            def u_pass(m, tag):
                ps = ps_s.tile([P, TQ], f32, tag="s", name=f"uB{m}")
                for kt in range(MT):
                    nc.tensor.matmul(ps[:], wsb["w1"][:, m, kt, :],
                                     z_sb[:, kt, :],
                                     start=(kt == 0), stop=(kt == MT - 1))
                return ps

            def f1_epi(m, ps):
                # hf = relu(rstd*U + nmr*(W1 g1) + (W1 b1 + bf1))
                t = tp.tile([P, TQ], bf16, tag="t")
                nc.vector.tensor_tensor(t[:], ps[:], a1_bf[:], OP.mult)
                nc.vector.scalar_tensor_tensor(
                    t[:], b1_bf[:], vec["w1g"][:, m:m + 1], t[:],
                    OP.mult, OP.add)
                nc.scalar.activation(hf[:, m, :], t[:], AF.Relu,
                                     bias=vec["wb1"][:, m:m + 1], scale=1.0)

            ub = {m: u_pass(m, "w") for m in (2, 3)}  # runs during stats
            rstd1, nmr1 = ln_stats(sum1_ps, sq1_ps, "g1", 0, TQ)
            a1_bf, b1_bf = make_reps(rstd1, nmr1, "g1", 0, TQ)
            f1_epi(0, psu[0])
            f1_epi(1, psu[1])
            ub[4] = u_pass(4, "s")
            ub[5] = u_pass(5, "s")
            f1_epi(2, ub[2])
            f1_epi(3, ub[3])
            ub[6] = u_pass(6, "w")
            ub[7] = u_pass(7, "w")
            for m in (4, 5, 6, 7):
                f1_epi(m, ub[m])

            # x1 = (xres - mu)*rstd*g1 + b1 (residual only; on Pool/Act)
            for mt in range(MT):
                u = tp.tile([P, TQ], bf16, tag="t")
                nc.gpsimd.tensor_tensor(u[:], xres[:, mt, :], a1_bf[:],
                                        OP.mult)
                nc.gpsimd.tensor_tensor(u[:], u[:], b1_bf[:], OP.add)
                nc.scalar.activation(x1[:, mt, :], u[:], AF.Identity,
                                     bias=vec["b1"][:, mt:mt + 1],
                                     scale=vec["g1"][:, mt:mt + 1])

            # ---- FFN2 + bias + residual -> yres, LN2 sums interleaved ----
            yres = pp.tile([P, MT, TQ], f32r, tag="v", name="yres")
            sum2_ps = ps_s.tile([1, TQ], f32, tag="s", name="lnsum_g2")
            sq2_ps = ps_s.tile([1, TQ], f32, tag="s", name="lnsq_g2")
            for mt in range(MT):
                if mt >= 1:
                    ln_accum(yres, mt - 1, sum2_ps, sq2_ps)
                ps = (ps_w.tile([P, TQ], f32, tag="w", name=f"f2_{mt}")
                      if mt % 2 else
                      ps_o.tile([P, TQ], f32, tag="o", name=f"f2_{mt}"))
                for kt in range(MT):
                    nc.tensor.matmul(ps[:], w2_sb[:, mt, kt, :],
                                     hf[:, kt, :],
                                     start=(kt == 0), stop=(kt == MT - 1))
                nc.vector.scalar_tensor_tensor(
                    yres[:, mt, :], ps[:], vec["bf2"][:, mt:mt + 1],
                    x1[:, mt, :], OP.add, OP.add)
            ln_accum(yres, 7, sum2_ps, sq2_ps)

            # ---- LN2 -> DRAM (f32), column-split tail ----
            # A(mt) = g2 x rstd and B(mt) = b2 x 1 + g2 x (-mu*rstd) as PE
            # outer products: normalize is then 2 tensor ops per tile.
            rows2 = {c0: ln_stats(sum2_ps, sq2_ps, f"g2c{c0}", c0, HTQ)
                     for c0 in (0, HTQ)}
            pair2 = None
            for c0 in (0, HTQ):
                pair2 = make_reps(*rows2[c0], f"g2c{c0}", c0, HTQ, pair2)
            reps2 = {0: pair2, HTQ: pair2}
            for c0 in (0, HTQ):
                a_bf, b_bf = reps2[c0]
                cs = slice(c0, c0 + HTQ)
                for mt in range(MT):
                    t = tp.tile([P, HTQ], f32r, tag="t")
                    nc.gpsimd.tensor_tensor(t[:], yres[:, mt, cs],
                                            a_bf[:, cs], OP.mult)
                    nc.vector.tensor_tensor(t[:], t[:], b_bf[:, cs], OP.add)
                    ot = op_pool.tile([P, HTQ], f32, tag="out", bufs=2,
                                      name=f"out{mt}_{c0}")
                    nc.scalar.activation(ot[:], t[:], AF.Identity,
                                         bias=vec["b2"][:, mt:mt + 1],
                                         scale=vec["g2"][:, mt:mt + 1])
                    (nc.sync if mt % 2 == 0 else nc.gpsimd).dma_start(
                        out_d[mt][:, cs], ot[:])

    nc.compile()
    return nc


def _np_bf16():
    import concourse.mybir as mybir
    return mybir.dt.np(mybir.dt.bfloat16)


def _safe_recip(g):
    g = g.astype(np.float64)
    return np.where(np.abs(g) < 1e-30, 1.0, 1.0 / g).astype(np.float32)


def _host_prep(inputs):
    """Gather valid kv columns, cast to bf16, tile-transpose."""
    bf16 = _np_bf16()
    mask = np.asarray(inputs["mask"])
    njv = int(max(int(np.ceil(int(mask[b].sum()) / P)) for b in range(B)))
    njv = max(njv, 1)
    nkvv = njv * P

    def xt(x):  # [n, DIM] f32 -> [P, MT, n] bf16
        return np.ascontiguousarray(
            x.T.reshape(MT, P, x.shape[0]).transpose(1, 0, 2)).astype(bf16)

    def wtiles(w):
        wt = w.T  # [k, m]
        return np.ascontiguousarray(
            wt.reshape(MT, P, MT, P).transpose(2, 1, 0, 3)).astype(bf16)

    def vecp(v):
        return np.ascontiguousarray(v.reshape(MT, P).T).astype(np.float32)

    xkT, xvT, maskbs = [], [], []
    for b in range(B):
        ib = np.nonzero(mask[b])[0]
        pad = nkvv - len(ib)
        idx = np.concatenate([ib, np.zeros(pad, ib.dtype)])
        mb = np.concatenate([np.zeros(len(ib), np.float32),
                             np.full(pad, NEG, np.float32)])
        maskbs.append(np.ascontiguousarray(mb.reshape(njv, P).T))
        xkT.append(xt(np.asarray(inputs["x_k"])[b][idx]))
        xvT.append(xt(np.asarray(inputs["x_v"])[b][idx]))

    host = {
        "njv": njv,
        "xkT": xkT,
        "xvT": xvT,
        "maskb": maskbs,
        "shared": {
            "onesr": np.ones((P, P), np.float32),
            "onesb": np.ones((P, P), _np_bf16()),
            "wv_r": np.ascontiguousarray(
                np.asarray(inputs["Wv"]).T.reshape(MT, P, DIM)
                .transpose(1, 0, 2)).astype(bf16),
            "wk": wtiles(np.asarray(inputs["Wk"])),
            "wq": wtiles(np.asarray(inputs["Wq"])),
            "wp": wtiles(np.asarray(inputs["Wp"])),
            "w1": wtiles(np.asarray(inputs["W1"])),
            "w2": wtiles(np.asarray(inputs["W2"])),
            "bp": vecp(np.asarray(inputs["bp"])),
            "w1g": vecp(np.asarray(inputs["W1"]).astype(np.float64)
                        @ np.asarray(inputs["g_ln1"]).astype(np.float64)),
            "wb1": vecp((np.asarray(inputs["W1"]).astype(np.float64)
                         @ np.asarray(inputs["b_ln1"]).astype(np.float64))
                        + np.asarray(inputs["bf1"]).astype(np.float64)),
            "bf1": vecp(np.asarray(inputs["bf1"])),
            "bf2": vecp(np.asarray(inputs["bf2"])),
            "g1": vecp(np.asarray(inputs["g_ln1"])),
            "rg1b": np.ascontiguousarray(
                _safe_recip(np.asarray(inputs["g_ln1"]))
                .reshape(MT, P).T).astype(bf16),
            "rg1sb": np.ascontiguousarray(
                np.square(_safe_recip(np.asarray(inputs["g_ln1"])))
                .reshape(MT, P).T).astype(bf16),
            "b1": vecp(np.asarray(inputs["b_ln1"])),
            "g2": vecp(np.asarray(inputs["g_ln2"])),
            "b2": vecp(np.asarray(inputs["b_ln2"])),
        },
    }
    return host


def _prep_core(inputs, b, qh, host):
    bf16 = _np_bf16()
    xq = np.asarray(inputs["x_q"])[b, qh * TQ:(qh + 1) * TQ, :]
    xqt = np.ascontiguousarray(xq.T.reshape(MT, P, TQ).transpose(1, 0, 2))
    d = {
        "xqT": xqt.astype(bf16),
        "xqTf": xqt.astype(np.float32),
        "xkT": host["xkT"][b],
        "xvT": host["xvT"][b],
        "maskb": host["maskb"][b],
    }
    d.update(host["shared"])
    return d


def get_nc(njv):
    key = ("nc", njv)
    if key not in _CACHE:
        _CACHE[key] = _build(njv)
    return _CACHE[key]


def kernel(**inputs):
    from concourse.bass_utils import run_bass_kernel_spmd
    inputs = {k: np.asarray(v) for k, v in inputs.items()}
    host = _host_prep(inputs)
    nc = get_nc(host["njv"])
    in_maps = []
    for c in range(8):
        in_maps.append(_prep_core(inputs, c // 2, c % 2, host))
    res = run_bass_kernel_spmd(nc, in_maps, list(range(8)))
    out = np.empty((B, NQ, DIM), np.float32)
    for c in range(8):
        b, qh = c // 2, c % 2
        oc = np.asarray(res.results[c]["out"]).astype(np.float32)  # [mt, p, q]
        out[b, qh * TQ:(qh + 1) * TQ, :] = (
            oc.transpose(2, 0, 1).reshape(TQ, DIM))
    return out

